# revision 30
# baseline (speedup 1.0000x reference)
"""Causal self-attention (B=2, T=2048, C=1024, 16 heads) on 8 trn2 cores.

Sharding: core = (batch b, head-group hg) on a 2x4 grid; each core computes
QKV projection, causal attention and the partial c_proj for its 4 heads of
one batch element. Host sums the 4 partials per batch element (replaces the
all-reduce) and adds bproj + bv@Wproj (the V-bias contribution is exact
because softmax rows sum to 1).

Device layout per core (all matmuls bf16):
  - x arrives host-prearranged chunk-major as xh [128, chunk=8, ck=8, 256]
    so each 256-column chunk is ONE dma (128 descriptors) and the first
    projection matmuls start ~2us in.  Weights arrive p-major so each is a
    single 128-descriptor dma.  DMA issue is split across sync (x, out,
    broadcasts) and vector (weights) queues.
  - K^T / Q^T produced as [128, T] bf16 tiles holding a HEAD-PAIR: head
    2hp at partitions 0..63, head 2hp+1 at 64..127.  One [128,512]
    tensor_scalar adds the bias (per-partition AP) and casts to bf16.
  - Scores: per k-tile, BOTH heads of a pair via two K=64 matmuls on
    disjoint PE row-groups (tile_position (0,0) / (64,0)) into one
    [128,1024] PSUM pair; they execute concurrently on the 128x128 array.
  - exp: ONE ScalarE activation per k-tile over [128, 2, 512-c] (both
    heads, ragged diagonal offset c shared).  Causal mask applied AFTER
    exp by one gpsimd affine_select per diagonal k-tile (both heads).
  - V stored [128, tt, h, 65] = per head [V(64) | 1]; AV accumulates
    O^T[65, q] so PSUM row 64 collects the softmax denominator.  M=65
    keeps LDWEIGHTS at 65 cols and needs no zero padding.
  - normalize: O^T copied to SBUF on gpsimd (frees the PSUM bank),
    exact DVE reciprocal on the [1,512] denominator row, ONE SBUF->SBUF
    broadcast dma [1,512]->[64,512], DVE multiply -> y^T bf16.  No DRAM
    round-trips.
  - c_proj per 128-row block: 4 matmuls -> bf16 SBUF -> one [128,1024]
    dma to HBM (bf16 partials, summed in f32 on host).
  - q-blocks processed 3,2,1,0 so the shortest dependency chain is last;
    proj work for block N drained during block N-1's attention.
"""

import sys
import types

import numpy as np

# ---------------------------------------------------------------------------
# Environment compatibility (self-contained on purpose).
# ---------------------------------------------------------------------------


def _install_axon_ntff_hook():
    """Provide the missing ``antenv.axon_hooks`` module so that
    ``run_bass_kernel_spmd(trace=True)`` works under axon in this container."""
    if "antenv.axon_hooks" in sys.modules:
        return
    try:
        import antenv
    except ImportError:
        return
    mod = types.ModuleType("antenv.axon_hooks")
    holder = [None]
    mod.set_axon_ntff_profile_hook = lambda h: holder.__setitem__(0, h)
    mod.get_axon_ntff_profile_hook = lambda: holder[0]
    sys.modules["antenv.axon_hooks"] = mod
    antenv.axon_hooks = mod
    try:
        from trn_agent_boot.trn_boot import _ntff_profile_via_ctypes

        hook = _ntff_profile_via_ctypes("/opt/axon/libaxon_pjrt.so")
        if hook is not None:
            mod.set_axon_ntff_profile_hook(hook)
    except Exception:
        pass


_install_axon_ntff_hook()

import concourse.bass as bass  # noqa: E402
import concourse.mybir as mybir  # noqa: E402
import concourse.tile as tile  # noqa: E402
from concourse.bass_utils import run_bass_kernel_spmd  # noqa: E402


def _split_multi_waits(nc, max_waits=1):
    """The walrus build here rejects instructions with more than one sync
    wait; move excess waits onto same-engine NoOps placed just before the
    instruction (sequential waiting is equivalent for monotonic sems)."""
    n = 0
    for func in nc.m.functions:
        for bb in func.blocks:
            out = []
            changed = False
            for inst in bb.instructions:
                si = inst.sync_info
                waits = list(si.on_wait) if si is not None and si.on_wait else []
                if len(waits) > max_waits:
                    changed = True
                    extra, keep = waits[:-max_waits], waits[-max_waits:]
                    for i in range(0, len(extra), max_waits):
                        n += 1
                        out.append(
                            mybir.InstNoOp(
                                name=f"{inst.name}-ws{i}",
                                engine=inst.engine,
                                ins=[],
                                outs=[],
                                sync_info=mybir.SyncInfo(
                                    on_wait=extra[i : i + max_waits], on_update=[]
                                ),
                                text_hint="wait_split",
                            )
                        )
                    si.on_wait = keep
                out.append(inst)
            if changed:
                bb.instructions = out
    return n


# ---------------------------------------------------------------------------
# Problem constants (hardcoded per spec).
# ---------------------------------------------------------------------------

B, T, C = 2, 2048, 1024
N_HEAD = 16
D = 64  # head dim
N_CORES = 8
HG = 4  # head groups (cores per batch element)
NH = N_HEAD // HG  # heads per core = 4
HP = NH // 2  # head pairs per core = 2
HD = NH * D  # head channels per core = 256
CK = C // 128  # contraction chunks = 8
TT = T // 128  # t tiles = 16
QB = T // 512  # q blocks = 4
NCH = 8  # x column chunks (256 cols each)

F32 = mybir.dt.float32
MM_DT = mybir.dt.bfloat16
MM_NP = mybir.dt.np(MM_DT)

TRACE = False
LAST_RESULT = None
_NC_CACHE = {}


def _build_nc():
    nc = bass.Bass("TRN2", target_bir_lowering=False)

    xh = nc.dram_tensor("xh", [128, NCH, CK, 256], MM_DT, kind="ExternalInput")
    wq = nc.dram_tensor("wq", [128, CK, HD], MM_DT, kind="ExternalInput")
    wk = nc.dram_tensor("wk", [128, CK, HD], MM_DT, kind="ExternalInput")
    wv = nc.dram_tensor("wv", [128, CK, HD], MM_DT, kind="ExternalInput")
    bias = nc.dram_tensor("bias", [128, 4], F32, kind="ExternalInput")
    wp = nc.dram_tensor("wp", [128, HD // 128, C], MM_DT, kind="ExternalInput")
    out = nc.dram_tensor("out", [T, C], MM_DT, kind="ExternalOutput")

    with tile.TileContext(nc) as tc:
        _emit(nc, tc, xh, wq, wk, wv, bias, wp, out)

    _split_multi_waits(nc)
    return nc


def _emit(nc, tc, xh, wq, wk, wv, bias, wp, out):
    from contextlib import ExitStack

    ctx = ExitStack()
    with ctx:
        consts = ctx.enter_context(tc.tile_pool(name="consts", bufs=1))
        qz_pool = ctx.enter_context(tc.tile_pool(name="qz", bufs=HP))
        kt_pool = ctx.enter_context(tc.tile_pool(name="kt", bufs=HD // 128))
        yt_pool = ctx.enter_context(tc.tile_pool(name="yt", bufs=2))
        pt_pool = ctx.enter_context(tc.tile_pool(name="pt", bufs=6))
        os_pool = ctx.enter_context(tc.tile_pool(name="os", bufs=4))
        rb_pool = ctx.enter_context(tc.tile_pool(name="rb", bufs=6))
        ob_pool = ctx.enter_context(tc.tile_pool(name="ob", bufs=3))
        dram = ctx.enter_context(tc.tile_pool(name="dram", bufs=3, space="DRAM"))
        # PSUM: st 2x2 banks + ot 2x1 + qk 2x1 = 8 banks
        p_qk = ctx.enter_context(tc.tile_pool(name="p_qk", bufs=2, space="PSUM"))
        p_st = ctx.enter_context(tc.tile_pool(name="p_st", bufs=2, space="PSUM"))
        p_ot = ctx.enter_context(tc.tile_pool(name="p_ot", bufs=2, space="PSUM"))

        # ---- loads ----------------------------------------------------------
        # x chunks on the sync queue; weights split in halves across the
        # scalar and gpsimd queues so the first K/V matmuls only wait for
        # ~256KB.  Each dma is a <=128-descriptor post.
        # warmup input first so the PE warmup can start immediately
        junk = consts.tile([128, 128], MM_DT, tag="junk")
        nc.gpsimd.memset(junk[:], 0.0)

        # x full chunks on sync (bandwidth-bound; chunks 6,7 first since
        # Q(qb=3) gates the first scores), wq halves on scalar, wk/wv
        # halves on gpsimd.
        wq_t = consts.tile([128, CK, HD], MM_DT, tag="wq")
        nc.scalar.dma_start(wq_t[:, 0:4], wq[:, 0:4])
        wk_t = consts.tile([128, CK, HD], MM_DT, tag="wk")
        nc.gpsimd.dma_start(wk_t[:, 0:4], wk[:, 0:4])
        nc.scalar.dma_start(wq_t[:, 4:8], wq[:, 4:8])
        nc.gpsimd.dma_start(wk_t[:, 4:8], wk[:, 4:8])

        xt = consts.tile([128, NCH, CK, 256], MM_DT, tag="xt")
        for c in (6, 7, 0, 1, 2, 3, 4, 5):
            nc.sync.dma_start(xt[:, c], xh[:, c])

        wv_t = consts.tile([128, CK, HD], MM_DT, tag="wv")
        nc.gpsimd.dma_start(wv_t[:, 0:4], wv[:, 0:4])
        nc.gpsimd.dma_start(wv_t[:, 4:8], wv[:, 4:8])
        bias_sb = consts.tile([128, 4], F32, tag="bias")
        nc.scalar.dma_start(bias_sb[:], bias[:])
        wp_t = consts.tile([128, HD // 128, C], MM_DT, tag="wp")
        nc.scalar.dma_start(wp_t[:], wp[:])

        # V tiles [128, tt, h, 65]: per head [V(64) | 1] so AV's psum row 64
        # accumulates the softmax denominator.  Ones column set once.
        vo = consts.tile([128, TT, NH, 65], MM_DT, tag="vo")
        nc.gpsimd.memset(
            vo[:].rearrange("p t h c -> p (t h) c")[:, :, D : D + 1], 1.0
        )

        # PE warmup while the first x chunks are in flight: ~3.5us of junk
        # matmuls lift the HAM clock gate to 8/8 before real work arrives,
        # so the QKV phase runs at 2.4GHz from the start.
        wps = p_qk.tile([128, 512], F32, tag="pq", name="warm")
        for _ in range(160):
            nc.tensor.matmul(wps[:, 0:64], junk[:], junk[:, 0:64],
                             start=True, stop=True)

        # constants for the PE-based tail normalize (last q-block): a ones
        # column, a ones row and a 64x64 identity (built in place).
        onep = consts.tile([128, 1], F32, tag="onep")
        nc.gpsimd.memset(onep[:], 1.0)
        oner = consts.tile([1, 64], F32, tag="oner")
        nc.gpsimd.memset(oner[:], 1.0)
        eye64 = consts.tile([64, 64], F32, tag="eye64")
        nc.gpsimd.memset(eye64[:], 1.0)
        nc.gpsimd.affine_select(
            out=eye64[:],
            in_=eye64[:],
            compare_op=mybir.AluOpType.is_equal,
            fill=0.0,
            base=0,
            pattern=[[1, 64]],
            channel_multiplier=-1,
        )

        # ---- QKV projection -------------------------------------------------
        qz_sb = [
            qz_pool.tile([128, T], MM_DT, tag="qz", name=f"qz{hp}")
            for hp in range(HP)
        ]
        kt_sb = [
            kt_pool.tile([128, T], MM_DT, tag="kt", name=f"kt{i}")
            for i in range(HD // 128)
        ]

        def emit_v(tt):
            c, half = tt // 2, tt % 2
            ps = p_qk.tile([128, 512], F32, tag="pq")
            for ck in range(CK):
                nc.tensor.matmul(
                    ps[:, :HD],
                    xt[:, c, ck, half * 128 : half * 128 + 128],
                    wv_t[:, ck, :],
                    start=(ck == 0),
                    stop=(ck == CK - 1),
                )
            nc.vector.tensor_copy(
                vo[:, tt, :, 0:D],
                ps[:, :HD].rearrange("p (h d) -> p h d", h=NH),
            )

        def emit_k(i, tb):
            ps = p_qk.tile([128, 512], F32, tag="pq")
            for ck in range(CK):
                nc.tensor.matmul(
                    ps[:],
                    wk_t[:, ck, i * 128 : (i + 1) * 128],
                    xt[:, 2 * tb : 2 * tb + 2, ck, :],
                    start=(ck == 0),
                    stop=(ck == CK - 1),
                )
            nc.vector.tensor_scalar(
                kt_sb[i][:, tb * 512 : (tb + 1) * 512],
                ps[:],
                bias_sb[:, 2 + i : 3 + i],
                None,
                mybir.AluOpType.add,
            )

        def emit_q(hp, tb):
            ps = p_qk.tile([128, 512], F32, tag="pq")
            for ck in range(CK):
                nc.tensor.matmul(
                    ps[:],
                    wq_t[:, ck, hp * 128 : (hp + 1) * 128],
                    xt[:, 2 * tb : 2 * tb + 2, ck, :],
                    start=(ck == 0),
                    stop=(ck == CK - 1),
                )
            nc.vector.tensor_scalar(
                qz_sb[hp][:, tb * 512 : (tb + 1) * 512],
                ps[:],
                bias_sb[:, hp : hp + 1],
                None,
                mybir.AluOpType.add,
            )

        # Only Q(qb=3), K(tb=0) and V(0,1) are produced up front (their x
        # chunks arrive first); the remaining K and V tiles are emitted
        # INSIDE the first q-block's kt loop as PE fillers, so exp starts
        # ~30us earlier than a separate QKV phase would allow.
        emit_q(0, 3)
        emit_q(1, 3)
        emit_k(0, 0)
        emit_k(1, 0)
        emit_v(0)
        emit_v(1)

        def qb3_fill(kt):
            if kt <= 13:
                emit_v(kt + 2)
            if kt in (1, 5, 9):
                tb = (kt + 3) // 4
                emit_k(0, tb)
                emit_k(1, tb)

        # ---- attention ------------------------------------------------------
        yt_sb = [
            yt_pool.tile([128, T], MM_DT, tag="yt", name=f"yt{g}")
            for g in range(HD // 128)
        ]

        # c_proj granules: one 128-row block of out per granule (4 matmuls,
        # 2 PSUM->SBUF bf16 copies, 1 dma); queued when a q-block's heads
        # finish, drained as PE fillers during the next q-block.
        proj_queue = []

        def make_proj(tt):
            def emit_proj():
                ob = ob_pool.tile([128, C], MM_DT, tag="ob")
                for nb in range(C // 512):
                    ps = p_qk.tile([128, 512], F32, tag="pq")
                    for g in range(HD // 128):
                        nc.tensor.matmul(
                            ps[:],
                            yt_sb[g][:, tt * 128 : (tt + 1) * 128],
                            wp_t[:, g, nb * 512 : (nb + 1) * 512],
                            start=(g == 0),
                            stop=(g == HD // 128 - 1),
                        )
                    if tt < 4:
                        nc.scalar.copy(ob[:, nb * 512 : (nb + 1) * 512], ps[:])
                    else:
                        nc.vector.tensor_copy(
                            ob[:, nb * 512 : (nb + 1) * 512], ps[:]
                        )
                nc.sync.dma_start(out[tt * 128 : (tt + 1) * 128, :], ob[:])

            return emit_proj

        def drain_proj(n):
            for _ in range(min(n, len(proj_queue))):
                proj_queue.pop(0)()

        def normalize_a(ot):
            # O^T rows to SBUF (frees the PSUM bank for the next head pair).
            o_sb = os_pool.tile([65, 512], F32, tag="os")
            nc.vector.tensor_copy(o_sb[:], ot[:])
            return o_sb

        def normalize_b(h, o_sb, q0):
            # denominator folded [1,512] -> [128,4] via DRAM (DVE reciprocal
            # is an 8-pass iterative op: [1,512] on one lane costs 3.3us,
            # [128,4] costs 0.17us), exact reciprocal, broadcast back via
            # DRAM; y^T = O^T * r in bf16.
            g, jb = h // 2, (h % 2) * 64
            rc_d = dram.tile([1, 512], F32, tag="rc_d")
            nc.sync.dma_start(rc_d[:], o_sb[64:65, :])
            r4 = rb_pool.tile([128, 4], F32, tag="r4")
            nc.sync.dma_start(r4[:], rc_d[0, :].rearrange("(p o) -> p o", p=128))
            nc.vector.reciprocal(r4[:], r4[:])
            rc2_d = dram.tile([1, 512], F32, tag="rc2_d")
            nc.sync.dma_start(rc2_d[0, :].rearrange("(p o) -> p o", p=128), r4[:])
            rb = rb_pool.tile([64, 512], F32, tag="rb")
            nc.sync.dma_start(rb[:], rc2_d[:].to_broadcast((64, 512)))
            nc.vector.tensor_tensor(
                yt_sb[g][jb : jb + 64, q0 : q0 + 512],
                o_sb[0:64, :],
                rb[:],
                mybir.AluOpType.mult,
            )

        def normalize_b_pe(h, o_sb, ot, q0):
            # all-on-chip variant for the LAST q-block (the PE is idle in
            # the tail, and the 4-dma chain's ~8us latency would be fully
            # exposed).  Fold den [1,512] -> [64,8] with eight K=1 matmuls,
            # DVE reciprocal, transpose back to a [1,512] row via eye64,
            # broadcast to [64,512] with a K=1 outer product, multiply.
            # All PSUM stages reuse the dead ot tile.
            g, jb = h // 2, (h % 2) * 64
            for j in range(8):
                nc.tensor.matmul(
                    ot[0:64, j : j + 1],
                    o_sb[64:65, 64 * j : 64 * j + 64],
                    onep[64:65, :],
                    start=True,
                    stop=True,
                )
            r8 = rb_pool.tile([64, 8], F32, tag="r8")
            nc.vector.reciprocal(r8[:], ot[0:64, 0:8])
            for j in range(8):
                nc.tensor.matmul(
                    ot[0:1, 64 * j : 64 * j + 64],
                    r8[:, j : j + 1],
                    eye64[:],
                    start=True,
                    stop=True,
                )
            rr = rb_pool.tile([1, 512], F32, tag="rr")
            nc.scalar.copy(rr[:], ot[0:1, :])
            nc.tensor.matmul(ot[0:64, :], oner[:], rr[:], start=True, stop=True)
            nc.vector.tensor_tensor(
                yt_sb[g][jb : jb + 64, q0 : q0 + 512],
                o_sb[0:64, :],
                ot[0:64, :],
                mybir.AluOpType.mult,
            )

        def emit_attention(qb):
            q0 = qb * 512
            n_kt = 4 * qb + 4

            for hp in range(HP):
                kd = kt_sb[hp]
                qd = qz_sb[hp]
                ots = {
                    jj: p_ot.tile([65, 512], F32, tag="ot", name=f"ot{hp}{jj}")
                    for jj in range(2)
                }

                def emit_av(kt, c, pt2):
                    for jj in range(2):
                        nc.tensor.matmul(
                            ots[jj][:, c:512],
                            vo[:, kt, 2 * hp + jj, :],
                            pt2[:, jj, c:512],
                            start=(kt == 0),
                            stop=(kt == n_kt - 1),
                        )

                pending = []
                for kt in range(n_kt):
                    j = kt - 4 * qb
                    c = 128 * j if j >= 0 else 0
                    # scores for BOTH heads: two K=64 matmuls on disjoint
                    # PE row-groups, executing concurrently.
                    st = p_st.tile([128, 1024], F32, tag="st")
                    st2 = st[:].rearrange("p (h q) -> p h q", h=2)
                    for jj in range(2):
                        nc.tensor.matmul(
                            st[:, jj * 512 + c : jj * 512 + 512],
                            kd[jj * 64 : jj * 64 + 64, kt * 128 : (kt + 1) * 128],
                            qd[jj * 64 : jj * 64 + 64, q0 + c : q0 + 512],
                            start=True,
                            stop=True,
                        )
                    pt = pt_pool.tile([128, 1024], MM_DT, tag="pt")
                    pt2 = pt[:].rearrange("p (h q) -> p h q", h=2)
                    nc.scalar.activation(
                        pt2[:, :, c:512],
                        st2[:, :, c:512],
                        mybir.ActivationFunctionType.Exp,
                        scale=0.125,
                    )
                    if j >= 0:
                        # causal mask: zero upper triangle of the diagonal
                        # 128-col window, both heads, after exp, on gpsimd.
                        nc.gpsimd.affine_select(
                            out=pt2[:, :, c : c + 128],
                            in_=pt2[:, :, c : c + 128],
                            compare_op=mybir.AluOpType.is_ge,
                            fill=0.0,
                            base=0,
                            pattern=[[0, 2], [1, 128]],
                            channel_multiplier=-1,
                        )
                    pending.append((kt, c, pt2))
                    if len(pending) > 1:
                        emit_av(*pending.pop(0))
                    if qb == 3 and hp == 0:
                        # remaining K/V tiles produced inside the first
                        # q-block's loop as PE fillers.
                        qb3_fill(kt)
                    if kt % 2 == 1:
                        drain_proj(1)
                    # Q projection for the next q-block, spread inside the
                    # kt loop so the PE burst never starves the exp conveyor.
                    if qb > 0 and kt == n_kt // 2:
                        emit_q(hp, qb - 1)
                for p in pending:
                    emit_av(*p)
                o_sbs = [normalize_a(ots[jj]) for jj in range(2)]
                for jj in range(2):
                    if qb == 0:
                        normalize_b_pe(2 * hp + jj, o_sbs[jj], ots[jj], q0)
                    else:
                        normalize_b(2 * hp + jj, o_sbs[jj], q0)
                drain_proj(1)

            # queue this q-block's c_proj row-blocks (drained during the
            # next q-block's attention; leftovers drained after the loop).
            for tt in range(qb * 4, qb * 4 + 4):
                proj_queue.append(make_proj(tt))

        # q-blocks descending: longest k-chain first, shortest last (small
        # serial tail).
        for qb in (3, 2, 1, 0):
            emit_attention(qb)
        drain_proj(len(proj_queue))


def _get_nc():
    key = str(MM_DT)
    if key not in _NC_CACHE:
        _NC_CACHE[key] = _build_nc()
    return _NC_CACHE[key]


def kernel(x, Wqkv, bqkv, Wproj, bproj):
    global LAST_RESULT
    x = np.asarray(x, dtype=np.float32)
    Wqkv = np.asarray(Wqkv, dtype=np.float32)
    bqkv = np.asarray(bqkv, dtype=np.float32)
    Wproj = np.asarray(Wproj, dtype=np.float32)
    bproj = np.asarray(bproj, dtype=np.float32)

    nc = _get_nc()
    in_maps = []
    for core in range(N_CORES):
        b, hg = core // HG, core % HG
        cs, ce = hg * HD, (hg + 1) * HD
        # x chunk-major: [p, chunk, ck, 256]
        xT = x[b].T  # [C, T] = [(ck p), t]
        xh = np.ascontiguousarray(
            xT.reshape(CK, 128, NCH, 256).transpose(1, 2, 0, 3).astype(MM_NP)
        )
        # weights p-major: [p, ck, n]
        def wslice(w):
            return np.ascontiguousarray(
                w.reshape(CK, 128, HD).transpose(1, 0, 2).astype(MM_NP)
            )

        bq = bqkv[cs:ce].reshape(HP, 128).T  # [128, HP]
        bk = bqkv[C + cs : C + ce].reshape(2, 128).T  # [128, 2]
        bias = np.ascontiguousarray(
            np.concatenate([bq, bk], axis=1).astype(np.float32)
        )
        in_maps.append(
            {
                "xh": xh,
                "wq": wslice(Wqkv[:, cs:ce]),
                "wk": wslice(Wqkv[:, C + cs : C + ce]),
                "wv": wslice(Wqkv[:, 2 * C + cs : 2 * C + ce]),
                "bias": bias,
                "wp": np.ascontiguousarray(
                    Wproj[cs:ce, :]
                    .reshape(HD // 128, 128, C)
                    .transpose(1, 0, 2)
                    .astype(MM_NP)
                ),
            }
        )

    res = run_bass_kernel_spmd(
        nc, in_maps, core_ids=list(range(N_CORES)), trace=TRACE
    )
    LAST_RESULT = res

    # V-bias contribution: y_true = y_dev + bv per head concat, and softmax
    # rows sum to exactly 1, so out += bv @ Wproj (host-side, exact).
    bv_full = bqkv[2 * C : 3 * C]
    bias_term = bv_full @ Wproj + bproj

    outp = np.empty((B, T, C), dtype=np.float32)
    for b in range(B):
        acc = res.results[b * HG]["out"].astype(np.float32)
        for hg in range(1, HG):
            acc = acc + res.results[b * HG + hg]["out"].astype(np.float32)
        outp[b] = acc + bias_term
    return outp
